# revision 23
# baseline (speedup 1.0000x reference)
"""Multi-head attention (ESIM-style masked softmax) on 8 trn2 NeuronCores.

Sharding: core c -> (batch b = c//2, head-group g = c%2). Each core runs
batch b with 8 of the 16 heads: Q/K/V projections restricted to its 512
channels (Wq/Wk/Wv column shards), attention, and a partial output
projection with its 512 rows of Wo. Host sums the two partials per batch.

On-device layout is fully "transposed" so no on-device transposes are
needed: host passes q^T/k^T/v^T; scores are computed as S^T[k,q]; the key
mask enters via zeroed V rows and a km-weighted denominator matmul; exp is
a single ACT op per tile; P^T feeds P·V directly as the moving operand.
All matmul operands are bf16 (weights loaded with FWL, half the HBM
traffic); accumulation stays fp32 in PSUM.
"""
import sys

for _p in ("/opt/trn_rl_repo",):
    if _p not in sys.path:
        sys.path.insert(0, _p)

import numpy as np

import concourse.bass as bass
import concourse.tile as tile
from concourse import mybir
from concourse.bass_utils import run_bass_kernel_spmd

# ---------------------------------------------------------------------------
# Workaround for this container's walrus build: it accepts at most ONE sem
# wait per lowered instruction. Split excess waits onto injected nops on the
# same (in-order) engine queue, and do the same for the kernel-tail drain.
# ---------------------------------------------------------------------------
import bass_rust
import concourse.tile as tile_mod
from concourse.vector_clock import ScopedClock

_MAX_WAITS = 1
_N_CARRIERS = 32
_wsplit_counter = [0]


def _patched_drain_and_barrier(self, tick_clock, wait_clock):
    nc = self.nc
    pre = [nc.sync.drain() for _ in range(_N_CARRIERS)]
    drain_inst = nc.sync.drain()
    wait_clock.add_sem_waits(
        drain_inst.ins, ScopedClock({None: tick_clock.global_clock})
    )
    si = drain_inst.ins.sync_info
    waits = list(si.on_wait) if si is not None else []
    if len(waits) > _MAX_WAITS:
        chunks = [waits[i : i + _MAX_WAITS] for i in range(0, len(waits), _MAX_WAITS)]
        *head, tail = chunks
        assert len(head) <= len(pre), f"too many drain waits: {len(waits)}"
        for inst, chunk in zip(pre, head):
            inst.ins.sync_info = bass_rust.SyncInfo(on_wait=chunk, on_update=[])
        drain_inst.ins.sync_info = bass_rust.SyncInfo(
            on_wait=tail, on_update=list(si.on_update) if si else []
        )
    nc.all_engine_barrier()
    assert self.sems is not None
    popped = nc._tile_sem_poison_stack.pop()
    assert popped is self._sem_poison
    nc.clear_and_free_semaphores(list(self.sems.allocated().values()))
    nc.all_engine_barrier()


def _split_excess_waits(nc, max_waits=_MAX_WAITS):
    n_split = 0
    for fn in nc.m.functions:
        for blk in fn.blocks:
            insts = blk.instructions
            if not any(
                inst.sync_info is not None
                and len(inst.sync_info.on_wait) > max_waits
                for inst in insts
            ):
                continue
            new = []
            for inst in insts:
                si = inst.sync_info
                waits = list(si.on_wait) if si is not None and si.on_wait else []
                if len(waits) > max_waits:
                    head, tail = waits[:-max_waits], waits[-max_waits:]
                    for w in head:
                        _wsplit_counter[0] += 1
                        nop = mybir.InstNoOp(
                            name=f"wsplit-{_wsplit_counter[0]}", ins=[], outs=[]
                        )
                        nop.engine = inst.engine
                        nop.sync_info = bass_rust.SyncInfo(on_wait=[w], on_update=[])
                        new.append(nop)
                        n_split += 1
                    inst.sync_info = bass_rust.SyncInfo(
                        on_wait=tail, on_update=list(si.on_update)
                    )
                new.append(inst)
            insts[:] = new
    return n_split


_orig_tile_exit = tile_mod.TileContext.__exit__


def _patched_tile_exit(self, *args, **kwargs):
    ret = _orig_tile_exit(self, *args, **kwargs)
    _split_excess_waits(self.nc)
    return ret


if getattr(tile_mod.TileContext, "_attn_patch", None) is None:
    tile_mod.TileContext._drain_and_barrier = _patched_drain_and_barrier
    tile_mod.TileContext.__exit__ = _patched_tile_exit
    tile_mod.TileContext._attn_patch = True

# ---------------------------------------------------------------------------
# Program constants
# ---------------------------------------------------------------------------
f32 = mybir.dt.float32
f32r = mybir.dt.float32r
bf16 = mybir.dt.bfloat16
AF = mybir.ActivationFunctionType
ALU = mybir.AluOpType

B, L, D = 4, 1024, 1024
CH = 512          # channels per core (8 heads x dh=64)
DC = 8            # d (contraction) chunks of 128
KC = 8            # key-position chunks of 128
LC = 8            # l (query/row) chunks of 128
N_CORES = 8
SCALE = 0.125     # 1/sqrt(dh)


def build_program():
    nc = bass.Bass(trn_type="TRN2", target_bir_lowering=False, debug=False)

    qT_d = nc.dram_tensor("qT", [D, L], bf16, kind="ExternalInput").ap()
    kT_d = nc.dram_tensor("kT", [D, L], bf16, kind="ExternalInput").ap()
    vT_d = nc.dram_tensor("vT", [D, L], bf16, kind="ExternalInput").ap()
    wq_d = nc.dram_tensor("wq", [D, CH], bf16, kind="ExternalInput").ap()
    wk_d = nc.dram_tensor("wk", [D, CH], bf16, kind="ExternalInput").ap()
    wv_d = nc.dram_tensor("wv", [D, CH], bf16, kind="ExternalInput").ap()
    wo_d = nc.dram_tensor("wo", [CH, D], bf16, kind="ExternalInput").ap()
    km_d = nc.dram_tensor("km", [128, KC], f32, kind="ExternalInput").ap()
    qm_d = nc.dram_tensor("qm", [128, LC], f32, kind="ExternalInput").ap()
    out_d = nc.dram_tensor("out", [L, D], bf16, kind="ExternalOutput").ap()

    with tile.TileContext(nc) as tc:
        with (
            tc.tile_pool(name="persist", bufs=1) as pers,
            tc.tile_pool(name="work", bufs=3) as work,
        ):
            # ---- persistent SBUF tiles ----
            wo_t = pers.tile([128, 4 * 1024], bf16, tag="wo")
            km_t = pers.tile([128, KC], f32, tag="km")
            qm_t = pers.tile([128, LC], f32, tag="qm")
            QT_t = pers.tile([128, 4 * 1024], bf16, tag="QT")
            KT_t = pers.tile([128, 4 * 1024], bf16, tag="KT")
            V_t = pers.tile([128, KC * 520], bf16, tag="V")
            vT_sb = pers.tile([128, DC * 1024], bf16, tag="vTsb")
            OT_ts = [pers.tile([128, 1024], bf16, tag=f"OT{i}", name=f"OT{i}")
                     for i in range(4)]

            nc.gpsimd.dma_start(km_t[:], km_d)
            nc.gpsimd.dma_start(qm_t[:], qm_d)

            def load_w(pool, dram):
                # allocate only; per-chunk DMAs are emitted interleaved with
                # the input-chunk DMAs so the first matmul starts early
                t = pool.tile([128, DC * 512], bf16,
                              tag=dram.tensor.name + "_t",
                              name=dram.tensor.name + "_t")
                return t

            # ---- K/Q projections (d-outer accumulation, 8 psum banks) ----
            with (
                tc.tile_pool(name="psP", bufs=8, space="PSUM") as psP,
                tc.tile_pool(name="pin", bufs=6) as pin,
                tc.tile_pool(name="wpool_qk", bufs=1) as wpool_qk,
            ):
                # KT = (k @ Wk)^T -> KT_t[c, l], c-chunk ci at cols ci*1024
                def proj_T(w_dram, x3, dst):
                    w_t = load_w(wpool_qk, w_dram)
                    w_s3 = w_dram.rearrange("(d p) n -> d p n", p=128)
                    psg = [psP.tile([128, 512], f32, tag="ps", name=f"psg{i}") for i in range(8)]
                    for d in range(DC):
                        nc.sync.dma_start(w_t[:, d * 512:(d + 1) * 512], w_s3[d])
                        x_c = pin.tile([128, L], bf16, tag="pin")
                        nc.sync.dma_start(x_c[:], x3[d])
                        for ci in range(4):
                            for lh in range(2):
                                nc.tensor.matmul(
                                    psg[ci * 2 + lh][:],
                                    w_t[:, d * 512 + ci * 128: d * 512 + (ci + 1) * 128],
                                    x_c[:, lh * 512:(lh + 1) * 512],
                                    start=(d == 0), stop=(d == DC - 1),
                                )
                    for ci in range(4):
                        for lh in range(2):
                            # alternate engines so the PSUM-bank WAR drains 2x
                            # faster before the V projection takes the banks
                            dsl = dst[:, ci * 1024 + lh * 512:
                                      ci * 1024 + (lh + 1) * 512]
                            if (ci * 2 + lh) % 2 == 0:
                                nc.vector.tensor_copy(dsl, psg[ci * 2 + lh][:])
                            else:
                                nc.scalar.activation(
                                    dsl, psg[ci * 2 + lh][:], AF.Copy)

                k3 = kT_d.rearrange("(d p) l -> d p l", p=128)
                q3 = qT_d.rearrange("(d p) l -> d p l", p=128)
                proj_T(wk_d, k3, KT_t)
                proj_T(wq_d, q3, QT_t)

            # V weights + full v^T resident (stationary for ki-outer V-proj)
            from contextlib import ExitStack
            _es = ExitStack()
            wpool_v = _es.enter_context(tc.tile_pool(name="wpool_v", bufs=1))
            wv_t = load_w(wpool_v, wv_d)
            wv_s3 = wv_d.rearrange("(d p) n -> d p n", p=128)
            v3 = vT_d.rearrange("(d p) l -> d p l", p=128)
            for d in range(DC):
                nc.sync.dma_start(wv_t[:, d * 512:(d + 1) * 512], wv_s3[d])
                nc.sync.dma_start(vT_sb[:, d * 1024:(d + 1) * 1024], v3[d])

            # ---- attention (+ V-projection interleaved into head 0) ----
            def st_exp(pool, h, qh, ki2):
                hp, ho = h // 2, (h % 2) * 64
                co = hp * 1024
                rows = slice(ho, ho + 64)
                qsl = slice(co + qh * 512, co + (qh + 1) * 512)
                st = pool.tile([128, 1024], f32, tag="st",
                               name=f"st_{h}_{qh}_{ki2}")
                for kk in range(2):
                    ki = 2 * ki2 + kk
                    ksl = slice(co + ki * 128, co + (ki + 1) * 128)
                    nc.tensor.matmul(
                        st[:, kk * 512:(kk + 1) * 512],
                        KT_t[rows, ksl], QT_t[rows, qsl],
                        start=True, stop=True,
                    )
                et = work.tile([128, 1024], bf16, tag="et",
                               name=f"et_{h}_{qh}_{ki2}")
                nc.scalar.activation(et[:], st[:], AF.Exp, scale=SCALE)
                return et

            def pv_mms(u, h, et, ki2):
                for kk in range(2):
                    ki = 2 * ki2 + kk
                    off = ki * 520 + h * 65
                    nc.tensor.matmul(
                        u[:], V_t[:, off:off + 65],
                        et[:, kk * 512:(kk + 1) * 512],
                        start=(ki == 0), stop=(ki == KC - 1),
                    )

            # post-PV: copy numerator + denominator out of PSUM right away
            # (frees the u bank in ~0.4us so PV never stalls); reciprocal is
            # batched per head on DVE; the reciprocal row is broadcast across
            # 64 partitions by a 1-contraction ones-matmul into PSUM (no DMA
            # round trip), and applied by a DVE multiply one head later.
            def stash(h, qh, u, dn):
                usb = normp.tile([64, 512], f32, tag="usb",
                                 name=f"usb_{h}_{qh}")
                nc.vector.tensor_copy(usb[:], u[0:64, :])
                # qh0 -> partition 0, qh1 -> partition 32 (DVE base-partition
                # restriction); the batched reciprocal is free-dim-bound so
                # the unused partitions in between are free
                nc.vector.tensor_copy(dn[32 * qh:32 * qh + 1, :], u[64:65, :])
                return usb

            def norm_recip(h, dn):
                # f32r output is bit-identical to f32 here; the dtype only
                # marks it as a legal fp32r-matmul operand for the broadcast
                rr = normp.tile([33, 512], f32r, tag="rr", name=f"rr_{h}")
                with nc.allow_low_precision(reason="f32r bits == f32"):
                    nc.vector.reciprocal(rr[:], dn[:])
                return rr

            def norm_apply(h, rr, usbs):
                hp, ho = h // 2, (h % 2) * 64
                rows = slice(ho, ho + 64)
                for qh in range(2):
                    qs2 = slice(qh * 512, (qh + 1) * 512)
                    rb = psB.tile([64, 512], f32, tag="rb",
                                  name=f"rb_{h}_{qh}")
                    nc.tensor.matmul(
                        rb[:],
                        ones_t[32 * qh:32 * qh + 1, :],
                        rr[32 * qh:32 * qh + 1, :],
                        start=True, stop=True,
                    )
                    nc.vector.tensor_tensor(
                        OT_ts[hp][rows, qs2], usbs[qh][:], rb[:], ALU.mult
                    )

            def vblock(ki, psv):
                blk = V_t[:, ki * 520:(ki + 1) * 520].rearrange(
                    "p (h c) -> p h c", c=65)
                nc.vector.tensor_scalar(
                    blk[:, :, 0:64],
                    psv[:].rearrange("p (h c) -> p h c", c=64),
                    km_t[:, ki:ki + 1], None, ALU.mult,
                )
                nc.vector.tensor_copy(
                    blk[:, :, 64:65],
                    km_t[:, ki:ki + 1][:, None, :].to_broadcast((128, 8, 1)),
                )

            with (
                tc.tile_pool(name="psA", bufs=1, space="PSUM") as psA,
                tc.tile_pool(name="normp", bufs=4) as normp,
            ):
                ones_f = pers.tile([33, 64], f32, tag="onesf")
                nc.vector.memset(ones_f[:], 1.0)
                ones_t = pers.tile([33, 64], f32r, tag="ones")
                nc.vector.tensor_copy(ones_t[:], ones_f[:])
                dn0 = work.tile([33, 512], f32, tag="dn", name="dn_0")
                # head 0 / qh 0, with V-projection chunks interleaved: the
                # ki-outer V-proj finishes V block ki right before PV needs it
                with (
                    tc.tile_pool(name="psV", bufs=2, space="PSUM") as psV,
                    tc.tile_pool(name="stA", bufs=2, space="PSUM") as stA,
                ):
                    u0 = psA.tile([65, 512], f32, tag="u", name="u_0_0")
                    for ki2 in range(4):
                        for kk in range(2):
                            ki = 2 * ki2 + kk
                            psv = psV.tile([128, 512], f32, tag="psv",
                                           name=f"psv_{ki}")
                            for d in range(DC):
                                nc.tensor.matmul(
                                    psv[:],
                                    vT_sb[:, d * 1024 + ki * 128:
                                          d * 1024 + (ki + 1) * 128],
                                    wv_t[:, d * 512:(d + 1) * 512],
                                    start=(d == 0), stop=(d == DC - 1),
                                )
                            vblock(ki, psv)
                        et = st_exp(stA, 0, 0, ki2)
                        pv_mms(u0, 0, et, ki2)
                    usb00 = stash(0, 0, u0, dn0)

                # wo is only needed by the output projection at the very end;
                # emit its loads after the critical K/Q/V input stream
                wo3 = wo_d.rearrange("(c p) n -> c p n", p=128)
                for ci in range(4):
                    nc.gpsimd.dma_start(
                        wo_t[:, ci * 1024:(ci + 1) * 1024], wo3[ci])

                with (
                    tc.tile_pool(name="stB", bufs=3, space="PSUM") as stB,
                    tc.tile_pool(name="psB", bufs=1, space="PSUM") as psB,
                ):
                    usbs = {(0, 0): usb00}
                    dns = {0: dn0}
                    pending = None  # (h, rr) whose broadcast+mult is deferred
                    for h in range(8):
                        dn = dns.get(h)
                        if dn is None:
                            dn = work.tile([33, 512], f32, tag="dn",
                                           name=f"dn_{h}")
                            dns[h] = dn
                        for qh in range(2):
                            if h == 0 and qh == 0:
                                continue
                            u = psA.tile([65, 512], f32, tag="u",
                                         name=f"u_{h}_{qh}")
                            for ki2 in range(4):
                                et = st_exp(stB, h, qh, ki2)
                                pv_mms(u, h, et, ki2)
                            usbs[(h, qh)] = stash(h, qh, u, dn)
                            if qh == 0 and pending is not None:
                                ph, prr = pending
                                norm_apply(ph, prr,
                                           [usbs[(ph, 0)], usbs[(ph, 1)]])
                                pending = None
                        rr = norm_recip(h, dn)
                        if h < 7:
                            pending = (h, rr)
                        else:
                            norm_apply(h, rr, [usbs[(h, 0)], usbs[(h, 1)]])

                # ---- output projection: PSUM-accumulate over the 4 head
                # pairs, then qm-scale (alternating Scalar/Vector) + DMA out
                with tc.tile_pool(name="psPP", bufs=4, space="PSUM") as psPP:
                    for li in range(LC):
                        for oh in range(2):
                            po = psPP.tile([128, 512], f32, tag="pp",
                                           name=f"po_{li}_{oh}")
                            for hp in range(4):
                                nc.tensor.matmul(
                                    po[:],
                                    OT_ts[hp][:, li * 128:(li + 1) * 128],
                                    wo_t[:, hp * 1024 + oh * 512:
                                         hp * 1024 + (oh + 1) * 512],
                                    start=(hp == 0), stop=(hp == 3),
                                )
                            ob = work.tile([128, 512], bf16, tag="ob",
                                           name=f"ob_{li}_{oh}")
                            if (li * 2 + oh) % 2 == 0:
                                nc.scalar.activation(
                                    ob[:], po[:], AF.Copy,
                                    scale=qm_t[:, li:li + 1],
                                )
                            else:
                                nc.vector.tensor_scalar(
                                    ob[:], po[:], qm_t[:, li:li + 1],
                                    None, ALU.mult,
                                )
                            nc.sync.dma_start(
                                out_d[li * 128:(li + 1) * 128,
                                      oh * 512:(oh + 1) * 512],
                                ob[:],
                            )
            _es.close()
    return nc


_cache = {}


def _get_program():
    if "nc" not in _cache:
        _cache["nc"] = build_program()
    return _cache["nc"]


BF16_NP = mybir.dt.np(bf16)


def build_in_maps(query, key, value, query_mask, key_mask, Wq, Wk, Wv, Wo):
    query = np.asarray(query, dtype=np.float32)
    key = np.asarray(key, dtype=np.float32)
    value = np.asarray(value, dtype=np.float32)
    Wq = np.asarray(Wq, dtype=np.float32)
    Wk = np.asarray(Wk, dtype=np.float32)
    Wv = np.asarray(Wv, dtype=np.float32)
    Wo = np.asarray(Wo, dtype=np.float32)

    qT = [np.ascontiguousarray(query[b].T).astype(BF16_NP) for b in range(B)]
    kT = [np.ascontiguousarray(key[b].T).astype(BF16_NP) for b in range(B)]
    vT = [np.ascontiguousarray(value[b].T).astype(BF16_NP) for b in range(B)]
    km = [
        np.ascontiguousarray(key_mask[b].astype(np.float32).reshape(KC, 128).T)
        for b in range(B)
    ]
    qm = [
        np.ascontiguousarray(query_mask[b].astype(np.float32).reshape(LC, 128).T)
        for b in range(B)
    ]
    wq_g = [np.ascontiguousarray(Wq[:, g * CH:(g + 1) * CH]).astype(BF16_NP)
            for g in range(2)]
    wk_g = [np.ascontiguousarray(Wk[:, g * CH:(g + 1) * CH]).astype(BF16_NP)
            for g in range(2)]
    wv_g = [np.ascontiguousarray(Wv[:, g * CH:(g + 1) * CH]).astype(BF16_NP)
            for g in range(2)]
    wo_g = [np.ascontiguousarray(Wo[g * CH:(g + 1) * CH, :]).astype(BF16_NP)
            for g in range(2)]

    in_maps = []
    for c in range(N_CORES):
        b, g = c // 2, c % 2
        in_maps.append({
            "qT": qT[b], "kT": kT[b], "vT": vT[b],
            "wq": wq_g[g], "wk": wk_g[g], "wv": wv_g[g], "wo": wo_g[g],
            "km": km[b], "qm": qm[b],
        })
    return in_maps


def kernel(query, key, value, query_mask, key_mask, Wq, Wk, Wv, Wo):
    nc = _get_program()
    in_maps = build_in_maps(query, key, value, query_mask, key_mask,
                            Wq, Wk, Wv, Wo)
    res = run_bass_kernel_spmd(nc, in_maps, list(range(N_CORES)))
    out = np.empty((B, L, D), dtype=np.float32)
    for b in range(B):
        out[b] = (res.results[2 * b]["out"].astype(np.float32)
                  + res.results[2 * b + 1]["out"].astype(np.float32))
    return out
